# revision 37
# baseline (speedup 1.0000x reference)
"""Expert-parallel grouped MLP (MoE routing) for Trainium2.

Problem: x[16384,1024] fp32, w1[8,1024,4096], w2[8,4096,1024],
rows_per_expert=2048.  out = gelu(x_e @ w1[e]) @ w2[e] per expert group.

Sharding: one expert per NeuronCore (E=8 == n_cores).  Each core runs an
identical Bass program on its own expert's slice; no collectives.  The host
pre-permutes each operand so every DMA chunk is a fully contiguous DRAM
region with 2-10KB per-partition lines:
    x  -> [NBLK, 128, HO, T_BLK]   (xp[b,p,h,ti]  = x[b*T_BLK+ti, h*128+p])
    w1 -> [FO, 128, H]             (w1p[f,p,h*128+fi] = w1[h*128+p, f*128+fi])
    w2 -> [HO, 128, F]             (w2p[h,p,f*128+hi] = w2[f*128+p, h*128+hi])
    boot-> [128, H + HO*T_BLK]     (w1 chunk 0 ++ all of x block 0)
    out <- [NBLK, HO, 128, T_BLK]  (out4[b,h,p,ti] = out[b*T_BLK+ti, h*128+p])
Activations stay in [feature, token] orientation through both GEMMs:
    GEMM1: interT[f,t] = sum_h w1[h,f] * xT[h,t]    (lhsT = w1 tile)
    gelu on PSUM -> SBUF (bf16)
    GEMM2: outT[h,t]  = sum_f w2[f,h] * interT[f,t]  (lhsT = w2 tile)
Matmuls run in bf16 (fp32 PSUM accumulate) - fp32 matmul is 4x slower on
the PE array.  Weights are SBUF-resident (64KB/partition each); tokens are
processed in 4 blocks of 512 so interT fits in SBUF.

The matmul stream runs at the PE issue floor (~215.8ns per N=512 bf16
matmul, LDWEIGHTS hidden), so the tuning below is all about the ends:
  - Startup: everything block-0 GEMM1 needs ships as ONE host-packed 1.25MB
    "boot" DMA (a single HWDGE queue processes DMAs serially at ~280GB/s;
    many thin triggers at ~0.65us each only reach ~190GB/s, and using the
    second (Scalar) HWDGE queue adds no bandwidth - shared backend).
    Remaining w1 chunks stream behind it, the later ones gated on compute
    progress so they never contend; dummy matmuls warm the PE clock (HAM
    needs a full ~3.4us busy window) until the boot DMA lands.
  - Tail: the last h-tile runs as a 384/128-token pair of PSUM chains so
    the big half's eviction overlaps the small half's matmuls - only 32KB
    drains after the final matmul.
"""

import numpy as np
import ml_dtypes

E = 8
H = 1024
F = 4096
T_PER_E = 2048
T_BLK = 512
NBLK = T_PER_E // T_BLK
P = 128
HO = H // P    # 8 contraction chunks for GEMM1
FO = F // P    # 32 contraction chunks for GEMM2
NW2 = 8        # w2 staged in HO chunks
NWARM = 26     # PE warm-up matmuls (N=256, cold ~213ns each)
NBOOT_X = 8    # xb0 h-chunks packed into the boot DMA (all of them)
W1_UNGATED = 5     # leading w1 chunks that stream immediately
W1_LOOKAHEAD = 5   # f-tiles of slack between a w1 chunk's DMA gate and its use

TRACE = False          # test.py sets kernel.TRACE = True for profiling
LAST_RESULTS = None    # BassKernelResults of the most recent run

_nc_cache = None


def _build_nc():
    import concourse.mybir as mybir
    import concourse.tile as tile
    from concourse import bacc
    from concourse.tile_rust import add_dep_helper

    bf16 = mybir.dt.bfloat16
    f32 = mybir.dt.float32
    GELU = mybir.ActivationFunctionType.Gelu_apprx_tanh

    nc = bacc.Bacc("TRN2", target_bir_lowering=False, debug=False)

    xp = nc.dram_tensor("xp", [NBLK, P, HO, T_BLK], bf16, kind="ExternalInput").ap()
    w1p = nc.dram_tensor("w1p", [FO, P, H], bf16, kind="ExternalInput").ap()
    w2p = nc.dram_tensor("w2p", [HO, P, F], bf16, kind="ExternalInput").ap()
    # Host-packed startup operands: per partition [w1 chunk 0 (H elems),
    # xb0 h0..h7 (NBOOT_X*T_BLK elems)].  One fat 1.25MB DMA delivers the
    # whole block-0 GEMM1 input set at the single-queue streaming rate
    # (~280GB/s, done ~12.8us), where the baseline's nine thin triggers were
    # serialized at ~0.65us each and finished only at ~17.7us.  (Measured:
    # splitting across the Sync+Scalar HWDGE queues does NOT add bandwidth —
    # the queues share the DMA backend.)
    BOOT = H + NBOOT_X * T_BLK
    bootp = nc.dram_tensor("bootp", [P, BOOT], bf16, kind="ExternalInput").ap()
    # Output in bf16: halves the store traffic draining at the kernel tail;
    # the host upcasts to fp32.  The added rounding (~1e-3 relative, on top
    # of the ~3.4e-3 from the bf16 matmuls) is negligible.
    out4 = nc.dram_tensor("out4", [NBLK, HO, P, T_BLK], bf16, kind="ExternalOutput").ap()

    with tile.TileContext(nc) as tc:
        with (
            tc.tile_pool(name="wpool", bufs=1) as wpool,
            tc.tile_pool(name="xpool", bufs=2) as xpool,
            tc.tile_pool(name="ipool", bufs=1) as ipool,
            tc.tile_pool(name="opool", bufs=3) as opool,
            tc.tile_pool(name="ps1", bufs=4, space="PSUM") as ps1,
            tc.tile_pool(name="ps2", bufs=4, space="PSUM") as ps2,
        ):
            # PE warm-up: dummy matmuls keep the PE busy while the first real
            # operands stream in, so the HAM clock gate reaches full rate
            # before the first real matmul.  The memset runs on Vector, whose
            # engine preamble finishes ~1.2us before GpSimd's, so the first
            # warm matmul isn't chained behind the slow GpSimd startup.
            warm = wpool.tile([P, 256], bf16, tag="warm")
            nc.vector.memset(warm[:], 0.0)
            for _ in range(NWARM):
                wp = ps2.tile([P, T_BLK], f32, tag="ps2t")
                nc.tensor.matmul(wp[:, 0:256], warm[:, 0:P], warm[:], start=True, stop=True)

            # w1 layout [P, FO, H]: lhsT for (h,f) = w1_sb[:, f, h*128:(h+1)*128]
            # w2 layout [P, HO, F]: lhsT for (f,h) = w2_sb[:, h, f*128:(f+1)*128]
            w1_sb = wpool.tile([P, FO, H], bf16, tag="w1sb")
            w2_sb = wpool.tile([P, HO, F], bf16, tag="w2sb")

            # All block-0 GEMM1 inputs arrive in the single boot DMA; the
            # remaining w1 chunks stream one 256KB trigger each behind it
            # (f0's 1.7us of matmuls covers w1[1]'s delivery).  Splitting
            # the boot into consumption-ordered slices was measured SLOWER
            # (per-DMA overhead/ramp resets: 223GB/s split vs ~300 single).
            # Block 0 reads f=0 weights and its x tiles straight out of
            # boot_sb (w1_sb chunk 0 is never loaded; blocks 1-3 also take
            # their f=0 weights from boot_sb).
            boot_sb = wpool.tile([P, BOOT], bf16, tag="boot")
            w1_dmas = [nc.sync.dma_start(boot_sb[:], bootp)]
            w1_dmas += [nc.sync.dma_start(w1_sb[:, f, :], w1p[f])
                        for f in range(1, FO)]
            w2_dmas = [
                nc.sync.dma_start(w2_sb[:, h, :], w2p[h]) for h in range(NW2)
            ]
            mm_first = {}   # (b, f) -> first matmul of that GEMM1 f-tile
            mm2_first = {}  # (b, h) -> first matmul of that GEMM2 h-tile
            xb_dmas = {}    # b -> prefetch DMA of block b's x tile

            def w1_tile(f, h):
                if f == 0:
                    return boot_sb[:, h * P:(h + 1) * P]
                return w1_sb[:, f, h * P:(h + 1) * P]

            for b in range(NBLK):
                if b == 0:
                    xb = None
                else:
                    xb = xpool.tile([P, HO, T_BLK], bf16, tag="xb")
                    xb_dmas[b] = nc.sync.dma_start(xb[:], xp[b])

                def x_tile(h, xb=xb):
                    if xb is None:
                        return boot_sb[:, H + h * T_BLK:H + (h + 1) * T_BLK]
                    return xb[:, h, :]

                it = ipool.tile([P, FO, T_BLK], bf16, tag="inter")
                for f in range(FO):
                    ps = ps1.tile([P, T_BLK], f32, tag="ps1t")
                    for h in range(HO):
                        mm = nc.tensor.matmul(
                            ps[:],
                            w1_tile(f, h),
                            x_tile(h),
                            start=(h == 0),
                            stop=(h == HO - 1),
                        )
                        if h == 0:
                            mm_first[(b, f)] = mm
                    nc.scalar.activation(it[:, f, :], ps[:], GELU)

                HB = T_BLK // 2
                for h in range(HO):
                    if not (b == NBLK - 1 and h == HO - 1):
                        ps = ps2.tile([P, T_BLK], f32, tag="ps2t")
                        for f in range(FO):
                            mm = nc.tensor.matmul(
                                ps[:],
                                w2_sb[:, h, f * P:(f + 1) * P],
                                it[:, f, :],
                                start=(f == 0),
                                stop=(f == FO - 1),
                            )
                            if f == 0:
                                mm2_first[(b, h)] = mm
                        # Evict in two halves so the DMA store of the first
                        # half overlaps the copy of the second.
                        ob = opool.tile([P, T_BLK], bf16, tag="ob")
                        nc.vector.tensor_copy(ob[:, :HB], ps[:, :HB])
                        nc.sync.dma_start(out4[b, h, :, :HB], ob[:, :HB])
                        nc.vector.tensor_copy(ob[:, HB:], ps[:, HB:])
                        nc.sync.dma_start(out4[b, h, :, HB:], ob[:, HB:])
                    else:
                        # Very last h-tile: run it as two independent
                        # accumulation chains over a 384/128 token split
                        # (separate PSUM banks; N-splitting a chain costs
                        # only the +2.5ns/MM issue overhead) so the big
                        # half's copy+store overlap the small half's matmuls
                        # -- only 32KB drains after the final matmul.
                        TA = 384
                        psA = ps2.tile([P, T_BLK], f32, tag="ps2t")
                        psB = ps2.tile([P, T_BLK], f32, tag="ps2t")
                        ob = opool.tile([P, T_BLK], bf16, tag="ob")
                        for f in range(FO):
                            mm = nc.tensor.matmul(
                                psA[:, 0:TA],
                                w2_sb[:, h, f * P:(f + 1) * P],
                                it[:, f, 0:TA],
                                start=(f == 0),
                                stop=(f == FO - 1),
                            )
                            if f == 0:
                                mm2_first[(b, h)] = mm
                        nc.vector.tensor_copy(ob[:, :TA], psA[:, 0:TA])
                        nc.sync.dma_start(out4[b, h, :, :TA], ob[:, :TA])
                        for f in range(FO):
                            nc.tensor.matmul(
                                psB[:, 0:T_BLK - TA],
                                w2_sb[:, h, f * P:(f + 1) * P],
                                it[:, f, TA:],
                                start=(f == 0),
                                stop=(f == FO - 1),
                            )
                        nc.vector.tensor_copy(ob[:, TA:], psB[:, 0:T_BLK - TA])
                        nc.sync.dma_start(out4[b, h, :, TA:], ob[:, TA:])

            # Stage every big load behind compute progress so chip-wide DMA
            # demand stays smooth (all 8 cores run this same program, so any
            # burst self-collides on HBM and starves the w1 stream -- seen
            # as 0.6-1.9us matmul gaps when w2's 8MB all fired in a 12us
            # window).  w1 chunk c waits for the f-tile W1_LOOKAHEAD tiles
            # ahead of its first consumer; w2 chunks spread one per h-tile
            # across GEMM2 block 0 (~10us of margin each); xb prefetches are
            # just-in-time (needed only at the NEXT block's GEMM1, 55-80us
            # after these gates fire).
            for c in range(W1_UNGATED, FO):
                add_dep_helper(
                    w1_dmas[c].ins, mm_first[(0, c - W1_LOOKAHEAD)].ins,
                    sync=True, reason="stage w1 load behind compute",
                )
            w2_gates = [mm_first[(0, 24)], mm_first[(0, 26)]] + [
                mm2_first[(0, c - 2)] for c in range(2, NW2)
            ]
            for c in range(NW2):
                add_dep_helper(
                    w2_dmas[c].ins, w2_gates[c].ins,
                    sync=True, reason="stage w2 load behind compute",
                )
            xb_gates = {1: mm2_first[(0, 2)], 2: mm_first[(1, 16)],
                        3: mm_first[(2, 16)]}
            for b2, gate in xb_gates.items():
                add_dep_helper(
                    xb_dmas[b2].ins, gate.ins,
                    sync=True, reason="stage x prefetch behind compute",
                )
    nc.compile()
    return nc


def _get_nc():
    global _nc_cache
    if _nc_cache is None:
        _nc_cache = _build_nc()
    return _nc_cache


def kernel(x, w1, w2, rows_per_expert):
    global LAST_RESULTS
    from concourse.bass_utils import run_bass_kernel_spmd

    x = np.asarray(x)
    w1 = np.asarray(w1)
    w2 = np.asarray(w2)
    rpe = int(rows_per_expert)
    assert x.shape == (E * rpe, H) and rpe == T_PER_E
    assert w1.shape == (E, H, F) and w2.shape == (E, F, H)

    bf16 = ml_dtypes.bfloat16
    in_maps = []
    for e in range(E):
        xe = x[e * rpe:(e + 1) * rpe].astype(bf16)      # [T, H]
        # [b*T_BLK+ti, ho*128+p] -> [b, p, ho, ti]
        xpm = np.ascontiguousarray(
            xe.reshape(NBLK, T_BLK, HO, P).transpose(0, 3, 2, 1)
        )
        # w1[ho*128+p, f*128+fi] -> [f, p, ho*128+fi]
        w1m = np.ascontiguousarray(
            w1[e].astype(bf16).reshape(HO, P, FO, P).transpose(2, 1, 0, 3)
        ).reshape(FO, P, H)
        # w2[fo*128+p, h*128+hi] -> [h, p, fo*128+hi]
        w2m = np.ascontiguousarray(
            w2[e].astype(bf16).reshape(FO, P, HO, P).transpose(2, 1, 0, 3)
        ).reshape(HO, P, F)
        # boot pack: per partition [w1 chunk 0 | xb0 h0..h{NBOOT_X-1}]
        bootm = np.concatenate(
            [w1m[0], xpm[0, :, :NBOOT_X, :].reshape(P, NBOOT_X * T_BLK)],
            axis=1,
        )
        bootm = np.ascontiguousarray(bootm)
        in_maps.append({"xp": xpm, "w1p": w1m, "w2p": w2m, "bootp": bootm})

    res = run_bass_kernel_spmd(_get_nc(), in_maps, list(range(E)), trace=TRACE)
    LAST_RESULTS = res

    out = np.empty((E * rpe, H), dtype=np.float32)
    for e in range(E):
        # [b, h, p, ti] -> [b*T_BLK+ti, h*128+p]
        o4 = res.results[e]["out4"].astype(np.float32)
        out[e * rpe:(e + 1) * rpe] = o4.transpose(0, 3, 1, 2).reshape(rpe, H)
    return out

